# revision 29
# baseline (speedup 1.0000x reference)
"""Multi-head attention (B=2, S=2048, D=1024, H=16) on 8 trn2 NeuronCores.

Sharding: core c handles batch b = c // 4 and head-group g = c % 4
(4 heads = 256 hidden columns per core).  Each core computes its 4 heads'
attention plus the partial out-projection; the host sums the 4 partials
per batch and adds the (linear) bias terms (bo + Wo @ bv) exactly.

All matmuls run in bf16 (1 cycle/row on the PE; fp32 accumulation in
PSUM).  Key structure, found via NTFF profiling:
  - activations are host-transposed to x.T so every PE contraction has
    d_model on partitions with line-rate contiguous DMA loads;
  - scores are computed transposed (scores_T = K_h^T-layout @ Q_h) so
    softmax's sum runs over PSUM partitions via a ones-column appended
    to V (denominator comes free out of the attnV matmul, row 64);
  - even/odd heads sit at partition base 0/64 so their K=64 score
    matmuls row-pack (run concurrently) in the PE array;
  - exp runs on ACT straight out of PSUM in [128,1024] tiles
    (scale=1/8 folded into the activation's affine input);
  - softmax normalization never touches the PE: reciprocal_approx_fast
    on DVE + partition_broadcast on the otherwise-idle GPSIMD;
  - the first head-pair's score/exp blocks are emitted before the V
    projection so the ACT exp pipe fills while the PE runs V;
  - out-projection consumes the transposed attention output directly
    (contraction over head columns), partials summed on host.

Measured on trn2 (8 cores): ~255 us HW exec, max rel err ~8e-3 vs
fp64 (bf16 input/weight rounding dominates; the f32r variant of this
kernel measured 4.7e-4 at ~2x the time — see kernel_f32r_v1.py).

Layout per core (DRAM tensors bf16 unless noted):
  xqT, xkT, xvT : [1024, 2048]   x.T (host-transposed activations)
  wqT, wkT, wvT : [1024, 256]    W.T column slice for this head group
  woT           : [256, 1024]    Wo[:, J].T
  bq, bk        : [256]  fp32    bias slices (added during PSUM evac)
  outT (output) : [1024, 2048] fp32   partial (out @ Wo_J.T).T
"""
import os
import sys
import types

sys.path.insert(0, "/opt/trn_rl_repo")

import numpy as np

B = 2
S = 2048
D = 1024
H_PER_CORE = 4      # heads per core
DH = 64             # head dim
JG = 256            # hidden cols per core (4 heads * 64)
ND = D // 128       # 8 contraction d-tiles
NKT = S // 128      # 16 k-position tiles
QC = 512
PC = 1024           # processed q columns per pass (2 chunks of 512)
SCALE = 1.0 / np.sqrt(DH)

_cache = {}


def _install_profshim():
    """Enable NTFF profiling under axon (KERNEL_TRACE=1 only)."""
    if "antenv.axon_hooks" in sys.modules:
        return
    try:
        from trn_agent_boot.trn_boot import _ntff_profile_via_ctypes

        hook = _ntff_profile_via_ctypes("/opt/axon/libaxon_pjrt.so")
        mod = types.ModuleType("antenv.axon_hooks")
        mod.get_axon_ntff_profile_hook = lambda: hook
        mod.set_axon_ntff_profile_hook = lambda h: None
        sys.modules["antenv.axon_hooks"] = mod
        import concourse.bass_utils as _bu

        _bu.upload_artifacts = lambda tmpdir: "local://unavailable"
    except Exception:
        pass


def build_nc():
    import concourse.bacc as bacc
    import concourse.mybir as mybir
    import concourse.tile as tile

    f32 = mybir.dt.float32
    bf16 = mybir.dt.bfloat16
    AF = mybir.ActivationFunctionType

    nc = bacc.Bacc("TRN2", target_bir_lowering=False)

    # weights arrive host-pre-arranged in the exact SBUF layout so every
    # load is one fully-contiguous DMA (4KB per partition row) instead of
    # a 512B-packet gather (which stalled the PE ~12us at kernel start)
    xqT = nc.dram_tensor("xqT", [D, S], bf16, kind="ExternalInput").ap()
    xkT = nc.dram_tensor("xkT", [D, S], bf16, kind="ExternalInput").ap()
    xvT = nc.dram_tensor("xvT", [D, S], bf16, kind="ExternalInput").ap()
    wqT = nc.dram_tensor("wqT", [128, ND * JG], bf16, kind="ExternalInput").ap()
    wkT = nc.dram_tensor("wkT", [128, ND * JG], bf16, kind="ExternalInput").ap()
    wvT = nc.dram_tensor("wvT", [128, ND * JG], bf16, kind="ExternalInput").ap()
    woT = nc.dram_tensor("woT", [128, 2 * ND * 128], bf16,
                         kind="ExternalInput").ap()
    bq = nc.dram_tensor("bq", [128, 2], f32, kind="ExternalInput").ap()
    bk = nc.dram_tensor("bk", [128, 2], f32, kind="ExternalInput").ap()
    outT = nc.dram_tensor("outT", [D, S], bf16, kind="ExternalOutput").ap()
    debug = int(os.environ.get("MHA_DEBUG", "0"))
    if debug:
        ao_dbg = [
            nc.dram_tensor(f"ao{m}_dbg", [128, S], bf16,
                           kind="ExternalOutput").ap()
            for m in range(2)
        ]
        v_dbg = nc.dram_tensor("v_dbg", [NKT, 128, H_PER_CORE * (DH + 1)],
                               bf16, kind="ExternalOutput").ap()

    with tile.TileContext(nc) as tc:
        with (
            tc.tile_pool(name="xt", bufs=15) as xt_pool,
            tc.tile_pool(name="wts", bufs=1) as w_pool,
            tc.tile_pool(name="qkv", bufs=1) as qkv_pool,
            tc.tile_pool(name="attn", bufs=1) as attn_pool,
            tc.tile_pool(name="small", bufs=1) as small_pool,
            tc.tile_pool(name="nrm", bufs=2) as nrm_pool,
            tc.tile_pool(name="oev", bufs=4) as oev_pool,
        ):
            # ---- weight / bias loads -------------------------------------
            # wq first so the very first projection matmul can start as
            # soon as wq + the first xq tile land; the rest follow behind
            wq_t = w_pool.tile([128, ND, JG], bf16, tag="wq")
            nc.sync.dma_start(wq_t[:], wqT.rearrange("p (n j) -> p n j", j=JG))
            wk_t = w_pool.tile([128, ND, JG], bf16, tag="wk")
            wv_t = w_pool.tile([128, ND, JG], bf16, tag="wv")
            wo_t = w_pool.tile([128, 2, ND, 128], bf16, tag="wo")
            bq_t = small_pool.tile([128, 2], f32, tag="bq")
            bk_t = small_pool.tile([128, 2], f32, tag="bk")
            ones1 = small_pool.tile([1, DH], f32, tag="ones1")
            nc.vector.memset(ones1[:], 1.0)

            def load_rest_of_weights():
                nc.sync.dma_start(
                    wk_t[:], wkT.rearrange("p (n j) -> p n j", j=JG)
                )
                nc.sync.dma_start(
                    wv_t[:], wvT.rearrange("p (n j) -> p n j", j=JG)
                )
                nc.sync.dma_start(
                    wo_t[:],
                    woT.rearrange("p (a n m) -> p a n m", a=2, m=128),
                )
                nc.sync.dma_start(bq_t[:], bq[:, :])
                nc.sync.dma_start(bk_t[:], bk[:, :])

            # ---- persistent activation tensors ---------------------------
            q_t = [qkv_pool.tile([128, S], bf16, tag=f"qt{m}", name=f"qt{m}")
                   for m in range(2)]
            k_t = [qkv_pool.tile([128, S], bf16, tag=f"kt{m}", name=f"kt{m}")
                   for m in range(2)]
            # V (natural layout) + ones column per head: 16 s-tiles
            v_t = [qkv_pool.tile([128, H_PER_CORE, DH + 1], bf16,
                                 tag=f"v{s}", name=f"v{s}")
                   for s in range(NKT)]
            ao_t = [qkv_pool.tile([128, S], bf16, tag=f"ao{m}", name=f"ao{m}")
                    for m in range(2)]

            # ---- phase 1: Q proj (both m) + K proj m=0 (4 PSUM banks) ----
            # Q-m1 runs in the xk DMA shadow.  xk and xv are loaded in
            # COLUMN-chunk order (c-outer) so K-m0's per-c accumulation
            # completes and evacuates progressively — the first score
            # quads (k cols 0:512) start ~20us before the full xk lands.
            xqs, xks = [], []
            with tc.tile_pool(name="proj_psum", bufs=1, space="PSUM") as pp:
                for d in range(ND):
                    xd = xt_pool.tile([128, S], bf16, tag="xT",
                                      name=f"xq{d}")
                    nc.sync.dma_start(xd[:], xqT[d * 128:(d + 1) * 128, :])
                    xqs.append(xd)
                nc.sync.dma_start(bq_t[:], bq[:, :])
                nc.sync.dma_start(bk_t[:], bk[:, :])
                nc.sync.dma_start(
                    wk_t[:], wkT.rearrange("p (n j) -> p n j", j=JG)
                )
                # xk in c-outer chunk order
                for d in range(ND):
                    xd = xt_pool.tile([128, S], bf16, tag="xT",
                                      name=f"xk{d}")
                    xks.append(xd)
                for c in range(4):
                    for d in range(ND):
                        nc.sync.dma_start(
                            xks[d][:, c * QC:(c + 1) * QC],
                            xkT[d * 128:(d + 1) * 128,
                                c * QC:(c + 1) * QC],
                        )
                # Q projection, both m halves (m1 fills the xk wait)
                for m in range(2):
                    ps = {
                        c: pp.tile([128, QC], f32, tag=f"pp{c}",
                                   name=f"psq{m}{c}")
                        for c in range(4)
                    }
                    for d in range(ND):
                        for c in range(4):
                            nc.tensor.matmul(
                                ps[c][:],
                                wq_t[:, d, m * 128:(m + 1) * 128],
                                xqs[d][:, c * QC:(c + 1) * QC],
                                start=(d == 0),
                                stop=(d == ND - 1),
                            )
                    for c in range(4):
                        nc.vector.tensor_scalar_add(
                            q_t[m][:, c * QC:(c + 1) * QC],
                            ps[c][:],
                            bq_t[:, m:m + 1],
                        )
                # remaining weights + xv (c-outer) behind xk
                nc.sync.dma_start(
                    wv_t[:], wvT.rearrange("p (n j) -> p n j", j=JG)
                )
                nc.sync.dma_start(
                    wo_t[:],
                    woT.rearrange("p (a n m) -> p a n m", a=2, m=128),
                )
                xvs = []
                for d in range(ND):
                    xd = xt_pool.tile([128, S], bf16, tag="xT",
                                      name=f"xv{d}")
                    xvs.append(xd)
                for c in range(4):
                    for d in range(ND):
                        nc.sync.dma_start(
                            xvs[d][:, c * QC:(c + 1) * QC],
                            xvT[d * 128:(d + 1) * 128,
                                c * QC:(c + 1) * QC],
                        )
                # K-m0, c-outer with progressive evacuation
                ps = {
                    c: pp.tile([128, QC], f32, tag=f"pp{c}",
                               name=f"psk0{c}")
                    for c in range(4)
                }
                for c in range(4):
                    for d in range(ND):
                        nc.tensor.matmul(
                            ps[c][:],
                            wk_t[:, d, 0:128],
                            xks[d][:, c * QC:(c + 1) * QC],
                            start=(d == 0),
                            stop=(d == ND - 1),
                        )
                    nc.vector.tensor_scalar_add(
                        k_t[0][:, c * QC:(c + 1) * QC],
                        ps[c][:],
                        bk_t[:, 0:1],
                    )

            # ---- phase 2+3: attention + V + out-projection ---------------
            # Paired-block pipeline: per (p, hp) block, the two heads'
            # score matmuls are emitted ADJACENTLY each kt with K=64
            # stationaries at partition base 0/64 (auto tile_position
            # (0,0)/(64,0)) so they genuinely row-pack in the PE.  Exp
            # tiles stay [128, PC] per head.  attnV runs one kt (or one
            # block) behind the exps; the V projection and the p=0
            # out-projection hide in the ACT exp gaps.  PSUM: sc0+sc1
            # (4 banks) + av0+av1 (4 banks, also lent to the V waves).
            with tc.tile_pool(name="apsum", bufs=1, space="PSUM") as ap_pool:
                ones4 = small_pool.tile([128, H_PER_CORE], f32, tag="ones4")
                nc.vector.memset(ones4[:], 1.0)

                def emit_sc_quad(p, hp, kt, ats_blk):
                    pc0 = p * PC
                    scs = []
                    for hh in range(2):
                        sc_ps = ap_pool.tile(
                            [128, PC], f32, tag=f"sc{hh}",
                            name=f"sc_{p}{hp}{hh}_{kt}",
                        )
                        scs.append(sc_ps)
                    # hh inner so the K=64 matmuls at bases 0/64 are
                    # adjacent in the PE queue -> run concurrently
                    if int(os.environ.get("MHA_PAIR", "1")):
                        order = [(n, hh) for n in range(2) for hh in range(2)]
                    else:
                        order = [(n, hh) for hh in range(2) for n in range(2)]
                    for n, hh in order:
                        po = hh * DH
                        nc.tensor.matmul(
                            scs[hh][:, n * QC:(n + 1) * QC],
                            k_t[hp][po:po + DH,
                                    kt * 128:(kt + 1) * 128],
                            q_t[hp][po:po + DH,
                                    pc0 + n * QC:pc0 + (n + 1) * QC],
                            start=True, stop=True,
                        )
                    for hh in range(2):
                        at = attn_pool.tile(
                            [128, PC], bf16, tag=f"at{hh}_{kt}",
                            name=f"at{p}{hp}{hh}_{kt}",
                        )
                        nc.scalar.activation(
                            at[:], scs[hh][:], AF.Exp, scale=float(SCALE)
                        )
                        ats_blk[(kt, hh)] = at

                def new_avs(p, hp):
                    return [
                        ap_pool.tile([DH + 1, PC], f32, tag=f"av{hh}",
                                     name=f"av{hh}_{hp}_{p}")
                        for hh in range(2)
                    ]

                def emit_av_kt(avs, hp, kt, ats_blk):
                    for hh in range(2):
                        h = hp * 2 + hh
                        at = ats_blk[(kt, hh)]
                        for n in range(2):
                            nc.tensor.matmul(
                                avs[hh][:, n * QC:(n + 1) * QC],
                                v_t[kt][:, h, :],
                                at[:, n * QC:(n + 1) * QC],
                                start=(kt == 0),
                                stop=(kt == NKT - 1),
                            )

                def norm_block(p, hp, hh, av):
                    # PE-free normalize.  Stage 1 (dn + av_sb copies) is
                    # all that touches the av PSUM, so the banks free after
                    # ~1us instead of the full recip/broadcast/mul chain —
                    # the next block's accumulator allocation waits only on
                    # stage 1.
                    psl = slice(p * PC, (p + 1) * PC)
                    po = hh * DH
                    dn = nrm_pool.tile([1, PC], f32, tag="dn",
                                       name=f"dn{p}{hp}{hh}")
                    nc.vector.tensor_copy(dn[:], av[DH:DH + 1, :])
                    av_sb = nrm_pool.tile([DH, PC], bf16, tag="avsb",
                                          name=f"avsb{p}{hp}{hh}")
                    nc.vector.tensor_copy(av_sb[:], av[0:DH, :])
                    rc = nrm_pool.tile([1, PC], f32, tag="rc",
                                       name=f"rc{p}{hp}{hh}")
                    nc.vector.reciprocal_approx_fast(rc[:], dn[:])
                    rb = nrm_pool.tile([DH, PC], f32, tag="rb",
                                       name=f"rb{p}{hp}{hh}")
                    nc.gpsimd.partition_broadcast(rb[:], rc[:])
                    nc.vector.tensor_mul(
                        ao_t[hp][po:po + DH, psl], av_sb[:], rb[:]
                    )

                def m1_proj_steps(which, d0, tiles, xs, w_full):
                    # two d-steps of the deferred m=1 half of Q/K proj.
                    # tiles = (pa, pb): av-slot PSUM [128, 2, QC] f32 —
                    # each c-region is its own bank (two per tile).
                    pa, pb = tiles
                    for d in (d0, d0 + 1):
                        for c in range(4):
                            reg = (pa if c < 2 else pb)[:, c % 2, :]
                            nc.tensor.matmul(
                                reg,
                                w_full[:, d, 128:256],
                                xs[d][:, c * QC:(c + 1) * QC],
                                start=(d == 0),
                                stop=(d == ND - 1),
                            )

                def m1_evac(which, tiles, dst, bias):
                    pa, pb = tiles
                    for c in range(4):
                        reg = (pa if c < 2 else pb)[:, c % 2, :]
                        nc.vector.tensor_scalar_add(
                            dst[1][:, c * QC:(c + 1) * QC],
                            reg,
                            bias[:, 1:2],
                        )

                def emit_v_wave(w):
                    # one wave = 2 s-tiles.  The two accumulators MUST sit
                    # in different PSUM banks (start=True resets at bank
                    # granularity), hence separate av0/av1 tags.
                    ps = {
                        si: ap_pool.tile([128, JG], f32, tag=f"av{si}",
                                         name=f"psv{2 * w + si}")
                        for si in range(2)
                    }
                    for d in range(ND):
                        for si in range(2):
                            s = 2 * w + si
                            nc.tensor.matmul(
                                ps[si][:],
                                xvs[d][:, s * 128:(s + 1) * 128],
                                wv_t[:, d, :],
                                start=(d == 0),
                                stop=(d == ND - 1),
                            )
                    for si in range(2):
                        s = 2 * w + si
                        nc.vector.tensor_copy(
                            v_t[s][:, :, 0:DH],
                            ps[si][:].rearrange("p (h d) -> p h d", d=DH),
                        )
                        nc.vector.tensor_copy(v_t[s][:, :, DH], ones4[:])

                def emit_wo_group(p, g, tag=None):
                    im, n = divmod(g, 2)
                    pc0 = p * PC
                    wo_ps = ap_pool.tile(
                        [128, QC], f32, tag=tag or f"sc{g % 2}",
                        name=f"wo{im}_{n}_{p}",
                    )
                    for jk in range(2):
                        nc.tensor.matmul(
                            wo_ps[:],
                            wo_t[:, jk, im, :],
                            ao_t[jk][:, pc0 + n * QC:pc0 + (n + 1) * QC],
                            start=(jk == 0),
                            stop=(jk == 1),
                        )
                    ot = oev_pool.tile([128, QC], bf16, tag="ot",
                                       name=f"ot{im}_{n}_{p}")
                    nc.vector.tensor_copy(ot[:], wo_ps[:])
                    nc.sync.dma_start(
                        outT[im * 128:(im + 1) * 128,
                             pc0 + n * QC:pc0 + (n + 1) * QC],
                        ot[:],
                    )

                # Driver.  Every slot emits its sc quad FIRST (so the exp
                # stream never waits behind filler work in the PE FIFO),
                # then the slot's filler: deferred m=1 projections and the
                # V waves in block 1, lagged attnV units + norms + wo
                # groups later.
                BLOCKS = [(0, 0), (0, 1), (1, 0), (1, 1)]
                ats = {}        # block index -> {(kt, hh): at tile}
                avs = {}        # block index -> [av_h0, av_h1]
                qm1 = km1 = None

                for bi, (p, hp) in enumerate(BLOCKS):
                    blk = {}
                    ats[bi] = blk
                    for kt in range(NKT):
                        emit_sc_quad(p, hp, kt, blk)
                        if bi == 0:
                            if kt == 0:
                                km1 = (
                                    ap_pool.tile([128, 2, QC], f32,
                                                 tag="av0", name="km1a"),
                                    ap_pool.tile([128, 2, QC], f32,
                                                 tag="av1", name="km1b"),
                                )
                            if kt < 4:
                                m1_proj_steps("k", 2 * kt, km1, xks, wk_t)
                            elif kt == 4:
                                m1_evac("k", km1, k_t, bk_t)
                            if 5 <= kt <= 12:
                                emit_v_wave(kt - 5)     # waves 0-7
                        elif bi == 1:
                            if kt == 0:
                                avs[0] = new_avs(0, 0)
                                emit_av_kt(avs[0], 0, 0, ats[0])
                            elif kt < 7:
                                emit_av_kt(avs[0], 0, 2 * kt - 1, ats[0])
                                emit_av_kt(avs[0], 0, 2 * kt, ats[0])
                            elif kt == 7:
                                emit_av_kt(avs[0], 0, 13, ats[0])
                            elif kt == 8:
                                emit_av_kt(avs[0], 0, 14, ats[0])
                                emit_av_kt(avs[0], 0, 15, ats[0])
                                for hh in range(2):
                                    norm_block(0, 0, hh, avs[0][hh])
                            elif kt == 9:
                                avs[1] = new_avs(0, 1)
                                emit_av_kt(avs[1], 1, 0, ats[1])
                                emit_av_kt(avs[1], 1, 1, ats[1])
                            else:
                                j = kt - 10
                                emit_av_kt(avs[1], 1, 2 * j + 2, ats[1])
                                emit_av_kt(avs[1], 1, 2 * j + 3, ats[1])
                        else:
                            prev = bi - 1
                            pp_, php = BLOCKS[prev]
                            if kt == 0:
                                if bi == 2:
                                    # finish block 2's attnV + its norm
                                    emit_av_kt(avs[1], 1, 14, ats[1])
                                    emit_av_kt(avs[1], 1, 15, ats[1])
                                else:
                                    emit_av_kt(avs[prev], php, 15,
                                               ats[prev])
                                for hh in range(2):
                                    norm_block(pp_, php, hh, avs[prev][hh])
                            elif kt == 1:
                                avs[bi] = new_avs(p, hp)
                                emit_av_kt(avs[bi], hp, 0, blk)
                            else:
                                emit_av_kt(avs[bi], hp, kt - 1, blk)
                            if kt >= 8:
                                emit_wo_group(0, (bi - 2) * 8 + kt - 8)

                # tail: last attnV unit, then a PE-assisted norm (the
                # reciprocal row is partition-broadcast by a K=1 ones
                # matmul into the freed sc slots — faster than GPSIMD and
                # keeps the PE warm), with the p=1 out-projection
                # interleaved at n granularity in the freed av slots.
                emit_av_kt(avs[3], 1, NKT - 1, ats[3])
                rbs = {}
                for hh in range(2):
                    av = avs[3][hh]
                    dn = nrm_pool.tile([1, PC], f32, tag="dn",
                                       name=f"dnt{hh}")
                    nc.vector.tensor_copy(dn[:], av[DH:DH + 1, :])
                    av_sb = nrm_pool.tile([DH, PC], bf16, tag="avsb",
                                          name=f"avsbt{hh}")
                    nc.vector.tensor_copy(av_sb[:], av[0:DH, :])
                    rc = nrm_pool.tile([1, PC], f32, tag="rc",
                                       name=f"rct{hh}")
                    nc.vector.reciprocal_approx_fast(rc[:], dn[:])
                    rb_ps = ap_pool.tile([DH, PC], f32, tag=f"sc{hh}",
                                         name=f"rbps{hh}")
                    for n in range(2):
                        nc.tensor.matmul(
                            rb_ps[:, n * QC:(n + 1) * QC],
                            ones1[0:1, :],
                            rc[0:1, n * QC:(n + 1) * QC],
                            start=True, stop=True,
                        )
                    rbs[hh] = (av_sb, rb_ps)
                for n in range(2):
                    for hh in range(2):
                        av_sb, rb_ps = rbs[hh]
                        nc.vector.tensor_mul(
                            ao_t[1][hh * DH:(hh + 1) * DH,
                                    PC + n * QC:PC + (n + 1) * QC],
                            av_sb[:, n * QC:(n + 1) * QC],
                            rb_ps[:, n * QC:(n + 1) * QC],
                        )
                    for im in range(ND):
                        emit_wo_group(1, im * 2 + n, tag=f"av{im % 2}")

                if debug:
                    for m in range(2):
                        nc.sync.dma_start(ao_dbg[m][:, :], ao_t[m][:, :])
                    for s in range(NKT):
                        nc.sync.dma_start(
                            v_dbg[s, :, :],
                            v_t[s].rearrange("p h d -> p (h d)"),
                        )

    nc.compile()
    return nc


def _enable_ldw_opt():
    """Let walrus dedupe consecutive identical LDWEIGHTS (off by default
    in concourse; our inner loops reuse each stationary operand 2-4x)."""
    if _cache.get("ldw_patched"):
        return
    import concourse.bass_utils as bu

    orig = bu.run_command

    def patched(argv, **kw):
        argv = [
            "--enable-ldw-opt=true" if a == "--enable-ldw-opt=false" else a
            for a in argv
        ]
        return orig(argv, **kw)

    bu.run_command = patched
    _cache["ldw_patched"] = True


def _get_nc():
    if "nc" not in _cache:
        # NOTE: --enable-ldw-opt=true crashes walrus codegen
        # (visitInstLdweights, CoreV3GenImpl.cpp:694) — keep off
        if int(os.environ.get("MHA_LDW_OPT", "0")):
            _enable_ldw_opt()
        _cache["nc"] = build_nc()
    return _cache["nc"]


def kernel(q, k, v, Wq, bq, Wk, bk, Wv, bv, Wo, bo, **_unused):
    import ml_dtypes
    from concourse.bass_utils import run_bass_kernel_spmd

    bf = ml_dtypes.bfloat16
    q = np.asarray(q, dtype=np.float32)
    k = np.asarray(k, dtype=np.float32)
    v = np.asarray(v, dtype=np.float32)
    Wq = np.asarray(Wq, dtype=np.float32)
    Wk = np.asarray(Wk, dtype=np.float32)
    Wv = np.asarray(Wv, dtype=np.float32)
    Wo = np.asarray(Wo, dtype=np.float32)
    bq = np.asarray(bq, dtype=np.float32)
    bk = np.asarray(bk, dtype=np.float32)
    bv = np.asarray(bv, dtype=np.float32)
    bo = np.asarray(bo, dtype=np.float32)

    nc = _get_nc()

    xT = {b: {} for b in range(B)}
    for b in range(B):
        xT[b]["q"] = np.ascontiguousarray(q[b].T).astype(bf)
        xT[b]["k"] = np.ascontiguousarray(k[b].T).astype(bf)
        xT[b]["v"] = np.ascontiguousarray(v[b].T).astype(bf)

    def _w_pre(wT_slice):
        # [1024, 256] -> [p=128, n=8, j=256] -> flat [128, 2048] contiguous
        return np.ascontiguousarray(
            wT_slice.reshape(ND, 128, JG).transpose(1, 0, 2).reshape(128, -1)
        ).astype(bf)

    wslices = []
    for g in range(4):
        J = slice(g * JG, (g + 1) * JG)
        wo_slice = Wo[:, J].T  # [256, 1024]
        wo_pre = np.ascontiguousarray(
            wo_slice.reshape(2, 128, ND, 128).transpose(1, 0, 2, 3)
            .reshape(128, -1)
        ).astype(bf)
        wslices.append({
            "wqT": _w_pre(np.ascontiguousarray(Wq.T[:, J])),
            "wkT": _w_pre(np.ascontiguousarray(Wk.T[:, J])),
            "wvT": _w_pre(np.ascontiguousarray(Wv.T[:, J])),
            "woT": wo_pre,
            "bq": np.ascontiguousarray(bq[J].reshape(2, 128).T),
            "bk": np.ascontiguousarray(bk[J].reshape(2, 128).T),
        })

    in_maps = []
    for c in range(8):
        b, g = c // 4, c % 4
        m = {
            "xqT": xT[b]["q"], "xkT": xT[b]["k"], "xvT": xT[b]["v"],
        }
        m.update(wslices[g])
        in_maps.append(m)

    trace = bool(int(os.environ.get("KERNEL_TRACE", "0")))
    if trace:
        _install_profshim()
    res = run_bass_kernel_spmd(
        nc, in_maps, core_ids=list(range(8)), trace=trace
    )
    _cache["exec_time_ns"] = res.exec_time_ns
    parts = [r["outT"] for r in res.results]

    # host reduce: sum the 4 head-group partials per batch (bf16 -> fp32),
    # transpose, add the linear bias terms (bo + Wo @ bv, exact fold)
    const_row = bo + Wo @ bv
    out = np.empty((B, S, D), dtype=np.float32)
    for b in range(B):
        acc = parts[4 * b].astype(np.float32)
        for g in range(1, 4):
            acc += parts[4 * b + g].astype(np.float32)
        out[b] = acc.T + const_row
    return out



# revision 32
# speedup vs baseline: 1.2317x; 1.2317x over previous
"""Multi-head attention (B=2, S=2048, D=1024, H=16) on 8 trn2 NeuronCores.

Sharding: core c handles batch b = c // 4 and head-group g = c % 4
(4 heads = 256 hidden columns per core).  Each core computes its 4 heads'
attention plus the partial out-projection; the host sums the 4 partials
per batch and adds the (linear) bias terms (bo + Wo @ bv) exactly.

All matmuls run in bf16 (1 cycle/row on the PE; fp32 accumulation in
PSUM).  Key structure, found via NTFF profiling:
  - activations are host-transposed to x.T so every PE contraction has
    d_model on partitions with line-rate contiguous DMA loads;
  - scores are computed transposed (scores_T = K_h^T-layout @ Q_h) so
    softmax's sum runs over PSUM partitions via a ones-column appended
    to V (denominator comes free out of the attnV matmul, row 64);
  - even/odd heads sit at partition base 0/64 so their K=64 score
    matmuls row-pack (run concurrently) in the PE array;
  - exp runs on ACT straight out of PSUM in [128,1024] tiles
    (scale=1/8 folded into the activation's affine input);
  - softmax normalization never touches the PE: reciprocal_approx_fast
    on DVE + partition_broadcast on the otherwise-idle GPSIMD;
  - the first head-pair's score/exp blocks are emitted before the V
    projection so the ACT exp pipe fills while the PE runs V;
  - out-projection consumes the transposed attention output directly
    (contraction over head columns), partials summed on host.

Measured on trn2 (8 cores): ~255 us HW exec, max rel err ~8e-3 vs
fp64 (bf16 input/weight rounding dominates; the f32r variant of this
kernel measured 4.7e-4 at ~2x the time — see kernel_f32r_v1.py).

Layout per core (DRAM tensors bf16 unless noted):
  xqT, xkT, xvT : [1024, 2048]   x.T (host-transposed activations)
  wqT, wkT, wvT : [1024, 256]    W.T column slice for this head group
  woT           : [256, 1024]    Wo[:, J].T
  bq, bk        : [256]  fp32    bias slices (added during PSUM evac)
  outT (output) : [1024, 2048] fp32   partial (out @ Wo_J.T).T
"""
import os
import sys
import types

sys.path.insert(0, "/opt/trn_rl_repo")

import numpy as np

B = 2
S = 2048
D = 1024
H_PER_CORE = 4      # heads per core
DH = 64             # head dim
JG = 256            # hidden cols per core (4 heads * 64)
ND = D // 128       # 8 contraction d-tiles
NKT = S // 128      # 16 k-position tiles
QC = 512
PC = 1024           # processed q columns per pass (2 chunks of 512)
SCALE = 1.0 / np.sqrt(DH)

_cache = {}


def _install_profshim():
    """Enable NTFF profiling under axon (KERNEL_TRACE=1 only)."""
    if "antenv.axon_hooks" in sys.modules:
        return
    try:
        from trn_agent_boot.trn_boot import _ntff_profile_via_ctypes

        hook = _ntff_profile_via_ctypes("/opt/axon/libaxon_pjrt.so")
        mod = types.ModuleType("antenv.axon_hooks")
        mod.get_axon_ntff_profile_hook = lambda: hook
        mod.set_axon_ntff_profile_hook = lambda h: None
        sys.modules["antenv.axon_hooks"] = mod
        import concourse.bass_utils as _bu

        _bu.upload_artifacts = lambda tmpdir: "local://unavailable"
    except Exception:
        pass


def build_nc():
    import concourse.bacc as bacc
    import concourse.mybir as mybir
    import concourse.tile as tile

    f32 = mybir.dt.float32
    bf16 = mybir.dt.bfloat16
    AF = mybir.ActivationFunctionType

    nc = bacc.Bacc("TRN2", target_bir_lowering=False)

    # weights arrive host-pre-arranged in the exact SBUF layout so every
    # load is one fully-contiguous DMA (4KB per partition row) instead of
    # a 512B-packet gather (which stalled the PE ~12us at kernel start)
    xqT = nc.dram_tensor("xqT", [D, S], bf16, kind="ExternalInput").ap()
    xkT = nc.dram_tensor("xkT", [D, S], bf16, kind="ExternalInput").ap()
    xvT = nc.dram_tensor("xvT", [D, S], bf16, kind="ExternalInput").ap()
    wqT = nc.dram_tensor("wqT", [128, ND * JG], bf16, kind="ExternalInput").ap()
    wkT = nc.dram_tensor("wkT", [128, ND * JG], bf16, kind="ExternalInput").ap()
    wvT = nc.dram_tensor("wvT", [128, ND * JG], bf16, kind="ExternalInput").ap()
    woT = nc.dram_tensor("woT", [128, 2 * ND * 128], bf16,
                         kind="ExternalInput").ap()
    bq = nc.dram_tensor("bq", [128, 2], f32, kind="ExternalInput").ap()
    bk = nc.dram_tensor("bk", [128, 2], f32, kind="ExternalInput").ap()
    outT = nc.dram_tensor("outT", [D, S], bf16, kind="ExternalOutput").ap()

    with tile.TileContext(nc) as tc:
        with (
            tc.tile_pool(name="xt", bufs=10) as xt_pool,
            tc.tile_pool(name="wts", bufs=1) as w_pool,
            tc.tile_pool(name="qkv", bufs=1) as qkv_pool,
            tc.tile_pool(name="attn", bufs=2) as attn_pool,
            tc.tile_pool(name="small", bufs=1) as small_pool,
            tc.tile_pool(name="nrm", bufs=3) as nrm_pool,
            tc.tile_pool(name="oev", bufs=4) as oev_pool,
        ):
            # ---- weight / bias loads -------------------------------------
            # wq first so the very first projection matmul can start as
            # soon as wq + the first xq tile land; the rest follow behind
            wq_t = w_pool.tile([128, ND, JG], bf16, tag="wq")
            nc.sync.dma_start(wq_t[:], wqT.rearrange("p (n j) -> p n j", j=JG))
            wk_t = w_pool.tile([128, ND, JG], bf16, tag="wk")
            wv_t = w_pool.tile([128, ND, JG], bf16, tag="wv")
            wo_t = w_pool.tile([128, 2, ND, 128], bf16, tag="wo")
            bq_t = small_pool.tile([128, 2], f32, tag="bq")
            bk_t = small_pool.tile([128, 2], f32, tag="bk")
            ones1 = small_pool.tile([1, DH], f32, tag="ones1")
            nc.vector.memset(ones1[:], 1.0)

            def load_rest_of_weights():
                nc.sync.dma_start(
                    wk_t[:], wkT.rearrange("p (n j) -> p n j", j=JG)
                )
                nc.sync.dma_start(
                    wv_t[:], wvT.rearrange("p (n j) -> p n j", j=JG)
                )
                nc.sync.dma_start(
                    wo_t[:],
                    woT.rearrange("p (a n m) -> p a n m", a=2, m=128),
                )
                nc.sync.dma_start(bq_t[:], bq[:, :])
                nc.sync.dma_start(bk_t[:], bk[:, :])

            # ---- persistent activation tensors ---------------------------
            q_t = [qkv_pool.tile([128, S], bf16, tag=f"qt{m}", name=f"qt{m}")
                   for m in range(2)]
            k_t = [qkv_pool.tile([128, S], bf16, tag=f"kt{m}", name=f"kt{m}")
                   for m in range(2)]
            # V (natural layout) + ones column per head: 16 s-tiles
            v_t = [qkv_pool.tile([128, H_PER_CORE, DH + 1], bf16,
                                 tag=f"v{s}", name=f"v{s}")
                   for s in range(NKT)]
            ao_t = [qkv_pool.tile([128, S], bf16, tag=f"ao{m}", name=f"ao{m}")
                    for m in range(2)]

            # ---- phase 1: projections (d-outer, PSUM-resident) -----------
            with tc.tile_pool(name="proj_psum", bufs=1, space="PSUM") as pp:
                for name, w_full, x_dram, dst, bias in (
                    ("q", wq_t, xqT, q_t, bq_t),
                    ("k", wk_t, xkT, k_t, bk_t),
                ):
                    xs = []
                    for d in range(ND):
                        xd = xt_pool.tile([128, S], bf16, tag="xT",
                                          name=f"x{name}{d}")
                        nc.sync.dma_start(
                            xd[:], x_dram[d * 128:(d + 1) * 128, :]
                        )
                        xs.append(xd)
                    if name == "q":
                        load_rest_of_weights()
                    ps = {
                        (m, c): pp.tile([128, QC], f32, tag=f"pp{m}{c}",
                                        name=f"ps{name}{m}{c}")
                        for m in range(2) for c in range(4)
                    }
                    for d in range(ND):
                        for m in range(2):
                            for c in range(4):
                                nc.tensor.matmul(
                                    ps[(m, c)][:],
                                    w_full[:, d, m * 128:(m + 1) * 128],
                                    xs[d][:, c * QC:(c + 1) * QC],
                                    start=(d == 0),
                                    stop=(d == ND - 1),
                                )
                    for m in range(2):
                        for c in range(4):
                            nc.vector.tensor_scalar_add(
                                dst[m][:, c * QC:(c + 1) * QC],
                                ps[(m, c)][:],
                                bias[:, m:m + 1],
                            )

                # xv tiles stream in behind the K loads
                xvs = []
                for d in range(ND):
                    xd = xt_pool.tile([128, S], bf16, tag="xT", name=f"xv{d}")
                    nc.sync.dma_start(xd[:], xvT[d * 128:(d + 1) * 128, :])
                    xvs.append(xd)

            # ---- phase 2+3: attention + V + out-projection ---------------
            # The first head-pair's score/exp blocks are emitted BEFORE the
            # V projection so the ACT exp queue drains while the PE runs V.
            with tc.tile_pool(name="apsum", bufs=1, space="PSUM") as ap_pool:

                def sc_exp_block(p, hp, hh):
                    pc0 = p * PC
                    po = hh * DH
                    ats = []
                    for kt in range(NKT):
                        sc_ps = ap_pool.tile(
                            [128, PC], f32, tag=f"sc{kt % 2}",
                            name=f"sc_{p}{hp}{hh}_{kt}",
                        )
                        for n in range(2):
                            nc.tensor.matmul(
                                sc_ps[:, n * QC:(n + 1) * QC],
                                k_t[hp][po:po + DH,
                                        kt * 128:(kt + 1) * 128],
                                q_t[hp][po:po + DH,
                                        pc0 + n * QC:pc0 + (n + 1) * QC],
                                start=True, stop=True,
                            )
                        at = attn_pool.tile(
                            [128, PC], bf16, tag=f"at{kt}",
                            name=f"at{p}{hp}{hh}_{kt}",
                        )
                        nc.scalar.activation(
                            at[:], sc_ps[:], AF.Exp, scale=float(SCALE)
                        )
                        ats.append(at)
                    return ats

                def av_block(p, hp, hh, ats):
                    h = hp * 2 + hh
                    av = ap_pool.tile(
                        [DH + 1, PC], f32, tag=f"av{hh}",
                        name=f"av{hh}_{hp}_{p}",
                    )
                    for kt in range(NKT):
                        for n in range(2):
                            nc.tensor.matmul(
                                av[:, n * QC:(n + 1) * QC],
                                v_t[kt][:, h, :],
                                ats[kt][:, n * QC:(n + 1) * QC],
                                start=(kt == 0),
                                stop=(kt == NKT - 1),
                            )
                    return av

                def norm_block(p, hp, hh, av):
                    # PE-free normalize: recip on DVE, partition broadcast
                    # on GPSIMD, multiply on DVE
                    psl = slice(p * PC, (p + 1) * PC)
                    po = hh * DH
                    dn = nrm_pool.tile([1, PC], f32, tag="dn",
                                       name=f"dn{p}{hp}{hh}")
                    nc.vector.tensor_copy(dn[:], av[DH:DH + 1, :])
                    rc = nrm_pool.tile([1, PC], f32, tag="rc",
                                       name=f"rc{p}{hp}{hh}")
                    nc.vector.reciprocal_approx_fast(rc[:], dn[:])
                    rb = nrm_pool.tile([DH, PC], f32, tag="rb",
                                       name=f"rb{p}{hp}{hh}")
                    nc.gpsimd.partition_broadcast(rb[:], rc[:])
                    nc.vector.tensor_mul(
                        ao_t[hp][po:po + DH, psl], av[0:DH, :], rb[:]
                    )

                # early scores for (p0, hp0) — fills the ACT pipe
                ats_early = {hh: sc_exp_block(0, 0, hh) for hh in range(2)}

                # V projection (PSUM slots borrowed from the av tags),
                # 8 waves of 2 s-tiles
                ones4 = small_pool.tile([128, H_PER_CORE], f32, tag="ones4")
                nc.vector.memset(ones4[:], 1.0)
                for w in range(8):
                    ps = {
                        s: ap_pool.tile([128, JG], f32, tag=f"av{s % 2}",
                                        name=f"psv{s}")
                        for s in (2 * w, 2 * w + 1)
                    }
                    for d in range(ND):
                        for s in (2 * w, 2 * w + 1):
                            nc.tensor.matmul(
                                ps[s][:],
                                xvs[d][:, s * 128:(s + 1) * 128],
                                wv_t[:, d, :],
                                start=(d == 0),
                                stop=(d == ND - 1),
                            )
                    for s in (2 * w, 2 * w + 1):
                        nc.vector.tensor_copy(
                            v_t[s][:, :, 0:DH],
                            ps[s][:].rearrange("p (h d) -> p h d", d=DH),
                        )
                        nc.vector.tensor_copy(v_t[s][:, :, DH], ones4[:])

                # (p0, hp0): attnV for the early heads, then normalize
                for hh in range(2):
                    av = av_block(0, 0, hh, ats_early[hh])
                    norm_block(0, 0, hh, av)

                def wo_block(p):
                    pc0 = p * PC
                    for im in range(ND):
                        for n in range(2):
                            wo_ps = ap_pool.tile(
                                [128, QC], f32, tag=f"sc{n}",
                                name=f"wo{im}_{n}_{p}",
                            )
                            for jk in range(2):
                                nc.tensor.matmul(
                                    wo_ps[:],
                                    wo_t[:, jk, im, :],
                                    ao_t[jk][:, pc0 + n * QC:
                                             pc0 + (n + 1) * QC],
                                    start=(jk == 0),
                                    stop=(jk == 1),
                                )
                            ot = oev_pool.tile([128, QC], bf16, tag="ot",
                                               name=f"ot{im}_{n}_{p}")
                            nc.vector.tensor_copy(ot[:], wo_ps[:])
                            nc.sync.dma_start(
                                outT[im * 128:(im + 1) * 128,
                                     pc0 + n * QC:pc0 + (n + 1) * QC],
                                ot[:],
                            )

                # remaining (p, hp) combos in standard order.  In the very
                # last block the attnV matmuls have to wait for their exps
                # (no later score block fills that PE gap), so the p=0
                # out-projection is slotted exactly there.
                for p, hp in ((0, 1), (1, 0), (1, 1)):
                    for hh in range(2):
                        ats = sc_exp_block(p, hp, hh)
                        if (p, hp, hh) == (1, 1, 1):
                            wo_block(0)
                        av = av_block(p, hp, hh, ats)
                        norm_block(p, hp, hh, av)
                wo_block(1)

    nc.compile()
    return nc


def _enable_ldw_opt():
    """Let walrus dedupe consecutive identical LDWEIGHTS (off by default
    in concourse; our inner loops reuse each stationary operand 2-4x)."""
    if _cache.get("ldw_patched"):
        return
    import concourse.bass_utils as bu

    orig = bu.run_command

    def patched(argv, **kw):
        argv = [
            "--enable-ldw-opt=true" if a == "--enable-ldw-opt=false" else a
            for a in argv
        ]
        return orig(argv, **kw)

    bu.run_command = patched
    _cache["ldw_patched"] = True


def _get_nc():
    if "nc" not in _cache:
        # NOTE: --enable-ldw-opt=true crashes walrus codegen
        # (visitInstLdweights, CoreV3GenImpl.cpp:694) — keep off
        if int(os.environ.get("MHA_LDW_OPT", "0")):
            _enable_ldw_opt()
        _cache["nc"] = build_nc()
    return _cache["nc"]


def kernel(q, k, v, Wq, bq, Wk, bk, Wv, bv, Wo, bo, **_unused):
    import ml_dtypes
    from concourse.bass_utils import run_bass_kernel_spmd

    bf = ml_dtypes.bfloat16
    q = np.asarray(q, dtype=np.float32)
    k = np.asarray(k, dtype=np.float32)
    v = np.asarray(v, dtype=np.float32)
    Wq = np.asarray(Wq, dtype=np.float32)
    Wk = np.asarray(Wk, dtype=np.float32)
    Wv = np.asarray(Wv, dtype=np.float32)
    Wo = np.asarray(Wo, dtype=np.float32)
    bq = np.asarray(bq, dtype=np.float32)
    bk = np.asarray(bk, dtype=np.float32)
    bv = np.asarray(bv, dtype=np.float32)
    bo = np.asarray(bo, dtype=np.float32)

    nc = _get_nc()

    xT = {b: {} for b in range(B)}
    for b in range(B):
        xT[b]["q"] = np.ascontiguousarray(q[b].T).astype(bf)
        xT[b]["k"] = np.ascontiguousarray(k[b].T).astype(bf)
        xT[b]["v"] = np.ascontiguousarray(v[b].T).astype(bf)

    def _w_pre(wT_slice):
        # [1024, 256] -> [p=128, n=8, j=256] -> flat [128, 2048] contiguous
        return np.ascontiguousarray(
            wT_slice.reshape(ND, 128, JG).transpose(1, 0, 2).reshape(128, -1)
        ).astype(bf)

    wslices = []
    for g in range(4):
        J = slice(g * JG, (g + 1) * JG)
        wo_slice = Wo[:, J].T  # [256, 1024]
        wo_pre = np.ascontiguousarray(
            wo_slice.reshape(2, 128, ND, 128).transpose(1, 0, 2, 3)
            .reshape(128, -1)
        ).astype(bf)
        wslices.append({
            "wqT": _w_pre(np.ascontiguousarray(Wq.T[:, J])),
            "wkT": _w_pre(np.ascontiguousarray(Wk.T[:, J])),
            "wvT": _w_pre(np.ascontiguousarray(Wv.T[:, J])),
            "woT": wo_pre,
            "bq": np.ascontiguousarray(bq[J].reshape(2, 128).T),
            "bk": np.ascontiguousarray(bk[J].reshape(2, 128).T),
        })

    in_maps = []
    for c in range(8):
        b, g = c // 4, c % 4
        m = {
            "xqT": xT[b]["q"], "xkT": xT[b]["k"], "xvT": xT[b]["v"],
        }
        m.update(wslices[g])
        in_maps.append(m)

    trace = bool(int(os.environ.get("KERNEL_TRACE", "0")))
    if trace:
        _install_profshim()
    res = run_bass_kernel_spmd(
        nc, in_maps, core_ids=list(range(8)), trace=trace
    )
    _cache["exec_time_ns"] = res.exec_time_ns
    parts = [r["outT"] for r in res.results]

    # host reduce: sum the 4 head-group partials per batch (bf16 -> fp32),
    # transpose, add the linear bias terms (bo + Wo @ bv, exact fold)
    const_row = bo + Wo @ bv
    out = np.empty((B, S, D), dtype=np.float32)
    for b in range(B):
        acc = parts[4 * b].astype(np.float32)
        for g in range(1, 4):
            acc += parts[4 * b + g].astype(np.float32)
        out[b] = acc.T + const_row
    return out

